# revision 1
# baseline (speedup 1.0000x reference)
"""Multi-head attention (B=2, L=2048, DIM=1024, 16 heads) on 8 trn2 cores.

Sharding: core = (batch b in 0..1) x (head-group hg in 0..3); each core
computes 4 heads of one batch element end-to-end (QKV proj, scores,
softmax, PV, partial out-proj). Host sums the 4 partial projections per
batch element and adds the bias.

Per-core layout strategy:
  - Q^T/K^T computed directly in [d, l] layout (w-stationary matmuls)
  - V computed in natural [l, d] layout with a ones column appended, so
    the PV matmul also produces the softmax denominator (row 64)
  - scores computed as S^T[j, i] (K^T as lhsT, Q^T as rhs), exp on ACT
  - O_unnorm^T[d, i] = V_aug^T @ E accumulated in PSUM over j
  - normalize with reciprocal + gpsimd partition_broadcast + DVE mult
  - out-proj consumes O_norm^T directly as lhsT (contraction over c)
All matmuls run as float32r (full-rate fp32 mode, fp32 data in memory).
"""

import ml_dtypes
import numpy as np

import bass_rust
import concourse.bass as bass
import concourse.tile as tile
from concourse import mybir
from concourse.bass_utils import run_bass_kernel_spmd
from concourse.vector_clock import ScopedClock

# ---- problem constants (hardcoded; kernel.py must be self-contained) ----
B = 2
L = 2048
DIM = 1024
NUM_HEADS = 16
HEAD_DIM = 64
SCALE = HEAD_DIM ** -0.5

NCORES = 8
NH = 4            # heads per core
C = NH * HEAD_DIM  # 256 head-cols per core
DA = HEAD_DIM + 1  # V augmented with ones column
KC = DIM // 128    # 8 contraction chunks for qkv proj
JC = L // 128      # 16 key-position chunks

F32 = mybir.dt.float32
F32R = mybir.dt.float32r
BF16 = mybir.dt.bfloat16

# walrus in this container rejects >4 sync waits on one CTRL (drain)
# instruction; split the final TileContext drain into multiple drains.
_MAX_DRAIN_WAITS = 1


def _wait_limit(inst):
    # walrus struct wait-slot capacity varies by opcode; matmul (S3_LW)
    # and DMA structs only fit one sync wait. Use 1 everywhere for safety.
    return 1


def _merge_waits(base, extra):
    """Merge sem waits; same-sem waits collapse to the max wait value."""
    out = {w.id: w for w in base}
    for w in extra:
        cur = out.get(w.id)
        if cur is None or w.wait_value > cur.wait_value:
            out[w.id] = w
    return list(out.values())


def _fix_excess_waits(nc):
    """Walrus encodes at most 1 sync wait per instruction in this build.
    For instructions carrying more, insert ENGINE_NOP wait-carriers
    immediately before them on the same engine stream — semantically
    identical (waits execute at the same stream position)."""
    def make_nop(like_inst):
        eng = nc.engines[like_inst.engine]
        bi = eng.nop(nofuse=True, hint="waitsplit")
        nop = bi.ins if hasattr(bi, "ins") else bi
        # isa() appended it to the current (last) block; pull it out.
        for bb2 in nc.main_func.blocks:
            lst = bb2.instructions
            if lst and lst[-1] is nop:
                lst.pop()
                break
        return nop

    for bb in nc.main_func.blocks:
        insts = bb.instructions  # live list
        i = 0
        while i < len(insts):
            inst = insts[i]
            si = inst.sync_info
            lim = _wait_limit(inst)
            if si is None or not si.on_wait or len(si.on_wait) <= lim:
                i += 1
                continue
            waits = _merge_waits(list(si.on_wait), [])
            if len(waits) <= lim:
                inst.sync_info = bass_rust.SyncInfo(
                    on_wait=waits, on_update=list(si.on_update)
                )
                i += 1
                continue
            keep = waits[-lim:]
            overflow = waits[:-lim]
            for w in overflow:
                nop = make_nop(inst)
                nop.sync_info = bass_rust.SyncInfo(on_wait=[w], on_update=[])
                insts.insert(i, nop)
                i += 1
            inst.sync_info = bass_rust.SyncInfo(
                on_wait=keep, on_update=list(si.on_update)
            )
            i += 1


def _split_drain_and_barrier(self, tick_clock, wait_clock):
    _fix_excess_waits(self.nc)
    drain_inst = self.nc.sync.drain()
    wait_clock.add_sem_waits(
        drain_inst.ins, ScopedClock({None: tick_clock.global_clock})
    )
    si = drain_inst.ins.sync_info
    waits = list(si.on_wait) if si is not None and si.on_wait else []
    if len(waits) > _MAX_DRAIN_WAITS:
        drain_inst.ins.sync_info = bass_rust.SyncInfo(
            on_wait=waits[:_MAX_DRAIN_WAITS], on_update=list(si.on_update)
        )
        rest = waits[_MAX_DRAIN_WAITS:]
        while rest:
            d2 = self.nc.sync.drain()
            d2.ins.sync_info = bass_rust.SyncInfo(
                on_wait=rest[:_MAX_DRAIN_WAITS], on_update=[]
            )
            rest = rest[_MAX_DRAIN_WAITS:]
    self.nc.all_engine_barrier()
    assert self.sems is not None
    popped = self.nc._tile_sem_poison_stack.pop()
    assert popped is self._sem_poison
    # RANGE_CLEAR's count field can't encode large ranges; clear in chunks.
    sems = list(self.sems.allocated().values())
    for k in range(0, len(sems), 8):
        self.nc.clear_and_free_semaphores(sems[k:k + 8])
    self.nc.all_engine_barrier()


tile.TileContext._drain_and_barrier = _split_drain_and_barrier

# This walrus build allows at most 2 sync waits per instruction. Collapse
# all HWDGE DMA completions onto a single semaphore lane so consumers that
# wait on two different DMAed tiles (plus a slot release) stay within the
# limit.
import concourse.tile_sem_assignment as _tsa  # noqa: E402

_tsa.NUM_HWDGE_SEMS = 1


def _build_nc() -> bass.Bass:
    nc = bass.Bass("TRN2", target_bir_lowering=False, debug=False)

    xT_h = nc.dram_tensor("xT", [DIM, L], BF16, kind="ExternalInput")
    wT_h = nc.dram_tensor("wT", [DIM, 3 * C], BF16, kind="ExternalInput")
    wpT_h = nc.dram_tensor("wpT", [C, DIM], BF16, kind="ExternalInput")
    ones_h = nc.dram_tensor("ones", [128, 64], BF16, kind="ExternalInput")
    onesr_h = nc.dram_tensor("onesr", [1, 64], F32R, kind="ExternalInput")
    out_h = nc.dram_tensor("out", [L, DIM], F32, kind="ExternalOutput")

    xT_r = xT_h[:].rearrange("(kc p) l -> p kc l", p=128)      # [128, 8, L]
    wT_r = wT_h[:].rearrange("(kc p) o -> p kc o", p=128)      # [128, 8, 768]
    wpT_r = wpT_h[:].rearrange("(cc p) o -> p cc o", p=128)    # [128, 2, 1024]

    with tile.TileContext(nc) as tc:
        import contextlib
        with contextlib.ExitStack() as ctx:
            singles = ctx.enter_context(tc.tile_pool(name="singles", bufs=1))
            xt_pool = ctx.enter_context(tc.tile_pool(name="xt", bufs=2))
            e_pool = ctx.enter_context(tc.tile_pool(name="e", bufs=6))
            small = ctx.enter_context(tc.tile_pool(name="small", bufs=2))
            out_pool = ctx.enter_context(tc.tile_pool(name="outp", bufs=4))

            wT_sb = singles.tile([128, KC, 3 * C], BF16)
            nc.sync.dma_start(wT_sb[:], wT_r)
            wpT_sb = singles.tile([128, 2, DIM], BF16)
            nc.sync.dma_start(wpT_sb[:], wpT_r)

            # Q^T/K^T in [o, l] layout: o in [0,512), head h at
            # chunk h//2, partition offset 64*(h%2); K at chunk 2+h//2.
            qkT_sb = singles.tile([128, 4, L], BF16)
            # V natural layout + ones col: v_sb[jp, jc, h, 0:64]=V, [...,64]=1
            # (ones DMAed from an input; f32r memset fails the ISA check)
            v_sb = singles.tile([128, JC, NH, DA], BF16)
            nc.sync.dma_start(
                v_sb[:, :, :, 64:65],
                ones_h[:].rearrange("p (a b) -> p a b", a=JC)[:, :, :, None],
            )
            # ones row for the recip broadcast matmul (K=1 outer product)
            ones_sb = singles.tile([1, 64], F32R)
            nc.sync.dma_start(ones_sb[:], onesr_h[:])
            # normalized O^T as lhsT for the out-proj; one tile per query
            # chunk so interleaved proj reads don't serialize against the
            # next chunk's normalize writes
            po_sbs = [singles.tile([128, 2, 1024], BF16, name=f"po{i}",
                                   tag=f"po{i}")
                      for i in range(2)]

            # ---- stage 1: QKV projections ----
            with tc.tile_pool(name="ps1", bufs=4, space="PSUM") as ps1:
                for lc in range(4):  # l-chunks of 512
                    xt = xt_pool.tile([128, KC, 512], BF16)
                    nc.sync.dma_start(xt[:], xT_r[:, :, lc * 512:(lc + 1) * 512])
                    for ot in range(4):  # Q,K out-tiles of 128
                        ps = ps1.tile([128, 512], F32, tag="ps1")
                        for kc in range(KC):
                            nc.tensor.matmul(
                                ps[:],
                                wT_sb[:, kc, ot * 128:(ot + 1) * 128],
                                xt[:, kc, :],
                                start=(kc == 0),
                                stop=(kc == KC - 1),
                            )
                        nc.vector.tensor_copy(
                            qkT_sb[:, ot, lc * 512:(lc + 1) * 512], ps[:]
                        )
                    for lt in range(4):  # V l-tiles of 128 within chunk
                        psv = ps1.tile([128, 256], F32, tag="ps1")
                        for kc in range(KC):
                            nc.tensor.matmul(
                                psv[:],
                                xt[:, kc, lt * 128:(lt + 1) * 128],
                                wT_sb[:, kc, 512:768],
                                start=(kc == 0),
                                stop=(kc == KC - 1),
                            )
                        jc = lc * 4 + lt
                        nc.vector.tensor_copy(
                            v_sb[:, jc, :, 0:64],
                            psv[:].rearrange("p (h d) -> p h d", h=NH),
                        )

            # ---- stage 2: attention + out-proj ----
            LAG = 2  # O runs this many j-chunks behind S/exp (no PE bubble)
            with tc.tile_pool(name="psS", bufs=2, space="PSUM") as psS_pool, \
                 tc.tile_pool(name="psO", bufs=2, space="PSUM") as psO_pool:

                def emit_proj_tile(pic, lt):
                    l0 = pic * 1024 + lt * 128
                    for oc in range(2):
                        psP = psO_pool.tile([128, 512], F32, tag="psO")
                        for cc in range(2):
                            nc.tensor.matmul(
                                psP[:],
                                po_sbs[pic][:, cc, lt * 128:(lt + 1) * 128],
                                wpT_sb[:, cc, oc * 512:(oc + 1) * 512],
                                start=(cc == 0),
                                stop=(cc == 1),
                            )
                        ot = out_pool.tile([128, 512], F32, tag="outp")
                        nc.vector.tensor_copy(ot[:], psP[:])
                        nc.sync.dma_start(
                            out_h[l0:l0 + 128, oc * 512:(oc + 1) * 512],
                            ot[:],
                        )

                proj_tasks = []
                for ic in range(2):  # query chunks of 1024
                    i0 = ic * 1024
                    po_sb = po_sbs[ic]
                    for h in range(NH):
                        pq = 64 * (h % 2)
                        cq = h // 2
                        psO = psO_pool.tile([DA, 1024], F32, tag="psO")
                        e_tiles = {}
                        for jc in range(JC + LAG):
                            if jc < JC:
                                psS = psS_pool.tile([128, 1024], F32, tag="psS")
                                for half in range(2):
                                    nc.tensor.matmul(
                                        psS[:, half * 512:(half + 1) * 512],
                                        qkT_sb[pq:pq + 64, 2 + cq,
                                               jc * 128:(jc + 1) * 128],
                                        qkT_sb[pq:pq + 64, cq,
                                               i0 + half * 512:i0 + (half + 1) * 512],
                                        start=True,
                                        stop=True,
                                    )
                                e = e_pool.tile([128, 1024], BF16, tag="e")
                                nc.scalar.activation(
                                    e[:], psS[:],
                                    mybir.ActivationFunctionType.Exp,
                                    scale=float(SCALE),
                                )
                                e_tiles[jc] = e
                            if jc >= LAG:
                                jo = jc - LAG
                                eo = e_tiles.pop(jo)
                                for half in range(2):
                                    nc.tensor.matmul(
                                        psO[:, half * 512:(half + 1) * 512],
                                        v_sb[:, jo, h, :],
                                        eo[:, half * 512:(half + 1) * 512],
                                        start=(jo == 0),
                                        stop=(jo == JC - 1),
                                    )
                        # normalize: rows 0:64 / row 64, into po_sb.
                        # Broadcast recip across partitions with a K=1
                        # matmul (ones[1,64]^T @ recip[1,1024]).
                        recip = small.tile([1, 1024], F32R, tag="recip")
                        with nc.allow_low_precision(reason="f32r same bits as f32"):
                            nc.vector.reciprocal(recip[:], psO[64:65, :])
                        rb = psS_pool.tile([64, 1024], F32, tag="psS")
                        for half in range(2):
                            nc.tensor.matmul(
                                rb[:, half * 512:(half + 1) * 512],
                                ones_sb[:],
                                recip[:, half * 512:(half + 1) * 512],
                                start=True,
                                stop=True,
                            )
                        # DVE can read only one PSUM operand: stage psO rows
                        # into po_sb, then scale in place against rb (PSUM).
                        nc.vector.tensor_copy(
                            po_sb[pq:pq + 64, cq, :], psO[0:64, :]
                        )
                        nc.vector.tensor_mul(
                            po_sb[pq:pq + 64, cq, :],
                            po_sb[pq:pq + 64, cq, :],
                            rb[:],
                        )
                        # fill ACT-bound stretches with pending proj work
                        for _ in range(3):
                            if proj_tasks:
                                emit_proj_tile(*proj_tasks.pop(0))
                    proj_tasks += [(ic, lt) for lt in range(8)]
                while proj_tasks:
                    emit_proj_tile(*proj_tasks.pop(0))
    return nc


_NC_CACHE = None


def _get_nc():
    global _NC_CACHE
    if _NC_CACHE is None:
        _NC_CACHE = _build_nc()
    return _NC_CACHE


def kernel(x, w_qkv, w_proj, b_proj, _trace=False):
    x = np.asarray(x, dtype=np.float32)
    w_qkv = np.asarray(w_qkv, dtype=np.float32)
    w_proj = np.asarray(w_proj, dtype=np.float32)
    b_proj = np.asarray(b_proj, dtype=np.float32)

    nc = _get_nc()
    in_maps = []
    for b in range(B):
        xT = np.ascontiguousarray(x[b].T)  # [DIM, L]
        for hg in range(4):
            s = C * hg
            wq = w_qkv[s:s + C]
            wk = w_qkv[DIM + s:DIM + s + C]
            wv = w_qkv[2 * DIM + s:2 * DIM + s + C]
            wT = np.ascontiguousarray(np.concatenate([wq, wk, wv], 0).T)
            wpT = np.ascontiguousarray(w_proj[:, s:s + C].T)
            in_maps.append({
                "xT": xT.astype(ml_dtypes.bfloat16),
                "wT": wT.astype(ml_dtypes.bfloat16),
                "wpT": wpT.astype(ml_dtypes.bfloat16),
                "ones": np.ones((128, 64), ml_dtypes.bfloat16),
                "onesr": np.ones((1, 64), np.float32),
            })

    res = run_bass_kernel_spmd(nc, in_maps, list(range(NCORES)), trace=_trace)
    parts = [res.results[i]["out"] for i in range(NCORES)]
    out = np.stack([
        parts[0] + parts[1] + parts[2] + parts[3],
        parts[4] + parts[5] + parts[6] + parts[7],
    ]).astype(np.float32) + b_proj[None, None, :].astype(np.float32)
    if _trace:
        return out, res
    return out



# revision 8
# speedup vs baseline: 1.4821x; 1.4821x over previous
"""Multi-head attention (B=2, L=2048, DIM=1024, 16 heads) on 8 trn2 cores.

Sharding: core = (batch b in 0..1) x (head-group hg in 0..3); each core
computes 4 heads of one batch element end-to-end (QKV proj, scores,
softmax, PV, partial out-proj). Host sums the 4 partial projections per
batch element and adds the bias.

v2 schedule (vs baseline):
  - heads processed as 2 PAIRS; the two heads of a pair occupy SBUF
    partition halves 0-63 / 64-127, so their K=64 score matmuls run
    CONCURRENTLY on disjoint PE row-groups (tile_position (0,0)/(64,0))
  - exp issued as one N=2048 ACTIVATE per 2 j-chunks (psS spans 4 PSUM
    banks) to amortize the ~352-cycle ACT instruction overhead
  - normalize: denominator rows staged to SBUF immediately (psO freed
    fast), one batched DVE reciprocal [2,512] per (pair, ic), one K=2
    matmul broadcasts both heads' 1/d to 128 partitions
  - pair-1 QKV and out-proj tiles interleaved as PE filler inside the
    ACT-bound attention loop; inputs DMAed in chunks so the first score
    matmul issues early
"""

import ml_dtypes
import numpy as np

import bass_rust
import concourse.bass as bass
import concourse.tile as tile
from concourse import mybir
from concourse.bass_utils import run_bass_kernel_spmd
from concourse.vector_clock import ScopedClock

# ---- problem constants (hardcoded; kernel.py must be self-contained) ----
B = 2
L = 2048
DIM = 1024
NUM_HEADS = 16
HEAD_DIM = 64
SCALE = HEAD_DIM ** -0.5

NCORES = 8
NH = 4             # heads per core
NPAIR = 2          # head pairs per core
C = NH * HEAD_DIM  # 256 head-cols per core
DA = HEAD_DIM + 1  # V augmented with ones column
KC = DIM // 128    # 8 contraction chunks for qkv proj
JC = L // 128      # 16 key-position chunks
NIC = 4            # query chunks of 512
ICW = 512          # query chunk width

F32 = mybir.dt.float32
F32R = mybir.dt.float32r
BF16 = mybir.dt.bfloat16

# walrus in this container rejects >4 sync waits on one CTRL (drain)
# instruction; split the final TileContext drain into multiple drains.
_MAX_DRAIN_WAITS = 1


def _wait_limit(inst):
    # walrus struct wait-slot capacity varies by opcode; matmul (S3_LW)
    # and DMA structs only fit one sync wait. Use 1 everywhere for safety.
    return 1


def _merge_waits(base, extra):
    """Merge sem waits; same-sem waits collapse to the max wait value."""
    out = {w.id: w for w in base}
    for w in extra:
        cur = out.get(w.id)
        if cur is None or w.wait_value > cur.wait_value:
            out[w.id] = w
    return list(out.values())


def _fix_excess_waits(nc):
    """Walrus encodes at most 1 sync wait per instruction in this build.
    For instructions carrying more, insert ENGINE_NOP wait-carriers
    immediately before them on the same engine stream — semantically
    identical (waits execute at the same stream position)."""
    def make_nop(like_inst):
        eng = nc.engines[like_inst.engine]
        bi = eng.nop(nofuse=True, hint="waitsplit")
        nop = bi.ins if hasattr(bi, "ins") else bi
        # isa() appended it to the current (last) block; pull it out.
        for bb2 in nc.main_func.blocks:
            lst = bb2.instructions
            if lst and lst[-1] is nop:
                lst.pop()
                break
        return nop

    for bb in nc.main_func.blocks:
        insts = bb.instructions  # live list
        i = 0
        while i < len(insts):
            inst = insts[i]
            si = inst.sync_info
            lim = _wait_limit(inst)
            if si is None or not si.on_wait or len(si.on_wait) <= lim:
                i += 1
                continue
            waits = _merge_waits(list(si.on_wait), [])
            if len(waits) <= lim:
                inst.sync_info = bass_rust.SyncInfo(
                    on_wait=waits, on_update=list(si.on_update)
                )
                i += 1
                continue
            keep = waits[-lim:]
            overflow = waits[:-lim]
            for w in overflow:
                nop = make_nop(inst)
                nop.sync_info = bass_rust.SyncInfo(on_wait=[w], on_update=[])
                insts.insert(i, nop)
                i += 1
            inst.sync_info = bass_rust.SyncInfo(
                on_wait=keep, on_update=list(si.on_update)
            )
            i += 1


def _split_drain_and_barrier(self, tick_clock, wait_clock):
    _fix_excess_waits(self.nc)
    drain_inst = self.nc.sync.drain()
    wait_clock.add_sem_waits(
        drain_inst.ins, ScopedClock({None: tick_clock.global_clock})
    )
    si = drain_inst.ins.sync_info
    waits = list(si.on_wait) if si is not None and si.on_wait else []
    if len(waits) > _MAX_DRAIN_WAITS:
        drain_inst.ins.sync_info = bass_rust.SyncInfo(
            on_wait=waits[:_MAX_DRAIN_WAITS], on_update=list(si.on_update)
        )
        rest = waits[_MAX_DRAIN_WAITS:]
        while rest:
            d2 = self.nc.sync.drain()
            d2.ins.sync_info = bass_rust.SyncInfo(
                on_wait=rest[:_MAX_DRAIN_WAITS], on_update=[]
            )
            rest = rest[_MAX_DRAIN_WAITS:]
    self.nc.all_engine_barrier()
    assert self.sems is not None
    popped = self.nc._tile_sem_poison_stack.pop()
    assert popped is self._sem_poison
    # RANGE_CLEAR's count field can't encode large ranges; clear in chunks.
    sems = list(self.sems.allocated().values())
    for k in range(0, len(sems), 8):
        self.nc.clear_and_free_semaphores(sems[k:k + 8])
    self.nc.all_engine_barrier()


tile.TileContext._drain_and_barrier = _split_drain_and_barrier

# This walrus build allows at most 2 sync waits per instruction. Collapse
# all HWDGE DMA completions onto a single semaphore lane so consumers that
# wait on two different DMAed tiles (plus a slot release) stay within the
# limit.
import concourse.tile_sem_assignment as _tsa  # noqa: E402

_tsa.NUM_HWDGE_SEMS = 1


def _build_nc() -> bass.Bass:
    nc = bass.Bass("TRN2", target_bir_lowering=False, debug=False)

    xT_h = nc.dram_tensor("xT", [DIM, L], BF16, kind="ExternalInput")
    # wT cols: [K_p0 | Q_p0 | V_p0 | K_p1 | Q_p1 | V_p1], 128 each
    wT_h = nc.dram_tensor("wT", [DIM, 3 * C], BF16, kind="ExternalInput")
    wpT_h = nc.dram_tensor("wpT", [C, DIM], BF16, kind="ExternalInput")
    ones_h = nc.dram_tensor("ones", [128, 32], BF16, kind="ExternalInput")
    onesr_h = nc.dram_tensor("onesr", [1, 64], F32, kind="ExternalInput")
    out_h = nc.dram_tensor("out", [L, DIM], F32, kind="ExternalOutput")

    xT_r = xT_h[:].rearrange("(kc p) l -> p kc l", p=128)      # [128, 8, L]
    wT_r = wT_h[:].rearrange("(kc p) o -> p kc o", p=128)      # [128, 8, 768]
    wpT_r = wpT_h[:].rearrange("(cc p) o -> p cc o", p=128)    # [128, 2, 1024]

    with tile.TileContext(nc) as tc:
        import contextlib
        with contextlib.ExitStack() as ctx:
            singles = ctx.enter_context(tc.tile_pool(name="singles", bufs=1))
            e_pool = ctx.enter_context(tc.tile_pool(name="e", bufs=3))
            dpool = ctx.enter_context(tc.tile_pool(name="dp", bufs=2))
            out_pool = ctx.enter_context(tc.tile_pool(name="outp", bufs=4))

            wT_sb = singles.tile([128, KC, 3 * C], BF16)
            wp_sb = singles.tile([128, 2, DIM], BF16)
            ones_sb = singles.tile([128, 32], BF16)
            onesr_sb = singles.tile([1, 64], F32)
            # x^T chunks: one tile per l-chunk of 512
            xt = [singles.tile([128, KC, 512], BF16, name=f"xt{lc}",
                               tag=f"xt{lc}") for lc in range(4)]
            # Q^T / K^T per pair: [128 (2 heads x 64 dims), L]
            qt = [singles.tile([128, L], BF16, name=f"qt{p}", tag=f"qt{p}")
                  for p in range(NPAIR)]
            kt = [singles.tile([128, L], BF16, name=f"kt{p}", tag=f"kt{p}")
                  for p in range(NPAIR)]
            # V natural layout + ones col, per pair
            vt = [singles.tile([128, JC, 2, DA], BF16, name=f"vt{p}",
                               tag=f"vt{p}") for p in range(NPAIR)]
            # normalized O^T as lhsT for the out-proj; per query chunk
            po = [singles.tile([128, NPAIR, ICW], BF16, name=f"po{i}",
                               tag=f"po{i}") for i in range(NIC)]

            # ---- input DMAs (sync HWDGE ring, FIFO): prefix needs
            # wT[K0] + xt0 first ----
            nc.sync.dma_start(wT_sb[:, :, 0:128], wT_r[:, :, 0:128])    # K0
            nc.sync.dma_start(xt[0][:], xT_r[:, :, 0:512])
            nc.sync.dma_start(wT_sb[:, :, 128:256], wT_r[:, :, 128:256])  # Q0
            nc.sync.dma_start(wT_sb[:, :, 256:384], wT_r[:, :, 256:384])  # V0
            nc.sync.dma_start(ones_sb[:], ones_h[:])
            nc.sync.dma_start(onesr_sb[:], onesr_h[:])
            nc.sync.dma_start(xt[1][:], xT_r[:, :, 512:1024])
            nc.sync.dma_start(wT_sb[:, :, 384:768], wT_r[:, :, 384:768])
            nc.sync.dma_start(xt[2][:], xT_r[:, :, 1024:1536])
            nc.sync.dma_start(xt[3][:], xT_r[:, :, 1536:2048])
            nc.sync.dma_start(wp_sb[:], wpT_r)

            # ones columns for V (denominator trick), via cheap DVE copy
            for p in range(NPAIR):
                nc.vector.tensor_copy(
                    vt[p][:, :, :, 64:65],
                    ones_sb[:].rearrange("q (a b) -> q a b", a=JC)[:, :, :, None],
                )

            with tc.tile_pool(name="psS", bufs=1, space="PSUM") as psS_pool, \
                 tc.tile_pool(name="psO", bufs=1, space="PSUM") as psO_pool, \
                 tc.tile_pool(name="scr", bufs=2, space="PSUM") as scr:

                # ---- stage-1 units (emitted inline or as filler) ----
                def kq_unit(p, lc, which):  # which: 0=Q, 1=K
                    off = p * 384 + (128 if which == 0 else 0)
                    ps = scr.tile([128, 512], F32, tag="scr")
                    for kc in range(KC):
                        nc.tensor.matmul(
                            ps[:],
                            wT_sb[:, kc, off:off + 128],
                            xt[lc][:, kc, :],
                            start=(kc == 0),
                            stop=(kc == KC - 1),
                        )
                    dst = qt[p] if which == 0 else kt[p]
                    nc.vector.tensor_copy(
                        dst[:, lc * 512:(lc + 1) * 512], ps[:]
                    )

                def v_unit(p, lc):
                    off = p * 384 + 256
                    for lt in range(4):
                        psv = scr.tile([128, 128], F32, tag="scr")
                        for kc in range(KC):
                            nc.tensor.matmul(
                                psv[:],
                                xt[lc][:, kc, lt * 128:(lt + 1) * 128],
                                wT_sb[:, kc, off:off + 128],
                                start=(kc == 0),
                                stop=(kc == KC - 1),
                            )
                        jc = lc * 4 + lt
                        nc.vector.tensor_copy(
                            vt[p][:, jc, :, 0:64],
                            psv[:].rearrange("q (hh d) -> q hh d", hh=2),
                        )

                def proj_tile(ic, it, oc):
                    psP = scr.tile([128, 512], F32, tag="scr")
                    for cc in range(2):
                        nc.tensor.matmul(
                            psP[:],
                            po[ic][:, cc, it * 128:(it + 1) * 128],
                            wp_sb[:, cc, oc * 512:(oc + 1) * 512],
                            start=(cc == 0),
                            stop=(cc == 1),
                        )
                    ot = out_pool.tile([128, 512], F32, tag="outp")
                    nc.vector.tensor_copy(ot[:], psP[:])
                    l0 = ic * ICW + it * 128
                    nc.sync.dma_start(
                        out_h[l0:l0 + 128, oc * 512:(oc + 1) * 512], ot[:]
                    )

                fq = []  # filler queue: closures emitting PE work

                def fill(n=1):
                    for _ in range(n):
                        if fq:
                            fq.pop(0)()

                # ---- prefix: pair-0 lc0 projections ----
                kq_unit(0, 0, 1)   # K0(lc0)
                kq_unit(0, 0, 0)   # Q0(lc0)
                v_unit(0, 0)       # V0(lc0)

                # psS: scores for 2 j-chunks x 2 heads -> one N=2048 exp
                psS = psS_pool.tile([128, 4, 512], F32, tag="psS")

                def attention_pair(p, on_normalize=None):
                    psO = [None, None]
                    e_tiles = {}

                    def scores_block(ic, t):
                        # j-chunks 2t, 2t+1; heads interleaved for PE
                        # row-group concurrency
                        for jj in (2 * t, 2 * t + 1):
                            for hh in range(2):
                                q = (jj % 2) * 2 + hh
                                nc.tensor.matmul(
                                    psS[:, q, :],
                                    kt[p][hh * 64:(hh + 1) * 64,
                                          jj * 128:(jj + 1) * 128],
                                    qt[p][hh * 64:(hh + 1) * 64,
                                          ic * ICW:(ic + 1) * ICW],
                                    start=True,
                                    stop=True,
                                )
                        e = e_pool.tile([128, 4, 512], BF16, tag="e")
                        nc.scalar.activation(
                            e[:], psS[:],
                            mybir.ActivationFunctionType.Exp,
                            scale=float(SCALE),
                        )
                        e_tiles[(ic, t)] = e

                    def pv_block(ic, t):
                        e = e_tiles.pop((ic, t))
                        for jj in (2 * t, 2 * t + 1):
                            for hh in range(2):
                                q = (jj % 2) * 2 + hh
                                nc.tensor.matmul(
                                    psO[hh][:],
                                    vt[p][:, jj, hh, :],
                                    e[:, q, :],
                                    start=(jj == 0),
                                    stop=(jj == JC - 1),
                                )

                    def normalize(ic):
                        # denominators of both heads into one [1, 1024]
                        # fp32 row; one fast approx reciprocal; two K=1
                        # matmuls broadcast 1/d to the partition halves.
                        d = dpool.tile([1, 1024], F32, tag="d")
                        for hh in range(2):
                            nc.vector.tensor_copy(
                                d[0:1, hh * 512:(hh + 1) * 512],
                                psO[hh][64:65, :],
                            )
                            nc.vector.tensor_copy(
                                po[ic][hh * 64:(hh + 1) * 64, p, :],
                                psO[hh][0:64, :],
                            )
                        rb = scr.tile([128, 512], F32, tag="scr")
                        for hh in range(2):
                            nc.tensor.matmul(
                                rb[hh * 64:(hh + 1) * 64, :],
                                onesr_sb[:],
                                d[0:1, hh * 512:(hh + 1) * 512],
                                start=True,
                                stop=True,
                            )
                        rr = dpool.tile([128, 512], F32, tag="r")
                        nc.vector.reciprocal(rr[:], rb[:])
                        nc.vector.tensor_mul(
                            po[ic][:, p, :], po[ic][:, p, :], rr[:]
                        )

                    for bt in range(NIC * 8 + 1):
                        ic, t = divmod(bt, 8)
                        if bt < NIC * 8:
                            if t == 0:
                                psO[0] = psO_pool.tile(
                                    [DA, 512], F32, name="psOA", tag="psOA")
                                psO[1] = psO_pool.tile(
                                    [DA, 512], F32, name="psOB", tag="psOB")
                            scores_block(ic, t)
                        if bt > 0:
                            pic, pt = divmod(bt - 1, 8)
                            pv_block(pic, pt)
                            if pt == 7:
                                normalize(pic)
                                if on_normalize is not None:
                                    on_normalize(pic)
                        fill(1)

                # ---- pair 0 attention; fillers finish p0 + all of p1
                # stage 1 (ordered by first-use) ----
                fq += [
                    lambda: kq_unit(0, 1, 1),    # K0(lc1): scores bt2
                    lambda: v_unit(0, 1),        # V0(lc1): pv bt3
                    lambda: kq_unit(0, 2, 1),    # K0(lc2): scores bt4
                    lambda: v_unit(0, 2),        # V0(lc2)
                    lambda: kq_unit(0, 3, 1),    # K0(lc3): scores bt6
                    lambda: v_unit(0, 3),        # V0(lc3)
                    lambda: kq_unit(0, 1, 0),    # Q0(lc1): ic1 = bt8
                    lambda: kq_unit(0, 2, 0),    # Q0(lc2)
                    lambda: kq_unit(0, 3, 0),    # Q0(lc3)
                    lambda: kq_unit(1, 0, 1),
                    lambda: kq_unit(1, 1, 1),
                    lambda: kq_unit(1, 2, 1),
                    lambda: kq_unit(1, 3, 1),
                    lambda: kq_unit(1, 0, 0),
                    lambda: v_unit(1, 0),
                    lambda: v_unit(1, 1),
                    lambda: v_unit(1, 2),
                    lambda: v_unit(1, 3),
                    lambda: kq_unit(1, 1, 0),
                    lambda: kq_unit(1, 2, 0),
                    lambda: kq_unit(1, 3, 0),
                ]
                attention_pair(0)
                while fq:
                    fill(1)

                # ---- pair 1 attention; fillers are proj tiles of
                # completed query chunks (legal once pair-1 normalize(ic)
                # has been emitted) ----
                def queue_proj(ic):
                    for it in range(4):
                        for oc in range(2):
                            fq.append(
                                lambda ic=ic, it=it, oc=oc:
                                    proj_tile(ic, it, oc)
                            )

                attention_pair(1, on_normalize=queue_proj)
                while fq:
                    fill(1)
    return nc


_NC_CACHE = None


def _get_nc():
    global _NC_CACHE
    if _NC_CACHE is None:
        _NC_CACHE = _build_nc()
    return _NC_CACHE


def kernel(x, w_qkv, w_proj, b_proj, _trace=False):
    x = np.asarray(x, dtype=np.float32)
    w_qkv = np.asarray(w_qkv, dtype=np.float32)
    w_proj = np.asarray(w_proj, dtype=np.float32)
    b_proj = np.asarray(b_proj, dtype=np.float32)

    nc = _get_nc()
    in_maps = []
    for b in range(B):
        xT = np.ascontiguousarray(x[b].T)  # [DIM, L]
        for hg in range(4):
            s = C * hg
            segs = []
            for p in range(NPAIR):
                o = s + p * 128
                segs.append(w_qkv[DIM + o:DIM + o + 128])      # K pair p
                segs.append(w_qkv[o:o + 128])                  # Q pair p
                segs.append(w_qkv[2 * DIM + o:2 * DIM + o + 128])  # V
            wT = np.ascontiguousarray(np.concatenate(segs, 0).T)
            wpT = np.ascontiguousarray(w_proj[:, s:s + C].T)
            in_maps.append({
                "xT": xT.astype(ml_dtypes.bfloat16),
                "wT": wT.astype(ml_dtypes.bfloat16),
                "wpT": wpT.astype(ml_dtypes.bfloat16),
                "ones": np.ones((128, 32), ml_dtypes.bfloat16),
                "onesr": np.ones((1, 64), np.float32),
            })

    res = run_bass_kernel_spmd(nc, in_maps, list(range(NCORES)), trace=_trace)
    parts = [res.results[i]["out"] for i in range(NCORES)]
    out = np.stack([
        parts[0] + parts[1] + parts[2] + parts[3],
        parts[4] + parts[5] + parts[6] + parts[7],
    ]).astype(np.float32) + b_proj[None, None, :].astype(np.float32)
    if _trace:
        return out, res
    return out


# revision 13
# speedup vs baseline: 1.6093x; 1.0858x over previous
"""Multi-head attention (B=2, L=2048, DIM=1024, 16 heads) on 8 trn2 cores.

Sharding: core = (batch b in 0..1) x (head-group hg in 0..3); each core
computes 4 heads of one batch element end-to-end (QKV proj, scores,
softmax, PV, partial out-proj). Host sums the 4 partial projections per
batch element and adds the bias.

v2 schedule (vs baseline):
  - heads processed as 2 PAIRS; the two heads of a pair occupy SBUF
    partition halves 0-63 / 64-127, so their K=64 score matmuls run
    CONCURRENTLY on disjoint PE row-groups (tile_position (0,0)/(64,0))
  - exp issued as one N=2048 ACTIVATE per 2 j-chunks (psS spans 4 PSUM
    banks) to amortize the ~352-cycle ACT instruction overhead
  - normalize: denominator rows staged to SBUF immediately (psO freed
    fast), one batched DVE reciprocal [2,512] per (pair, ic), one K=2
    matmul broadcasts both heads' 1/d to 128 partitions
  - pair-1 QKV and out-proj tiles interleaved as PE filler inside the
    ACT-bound attention loop; inputs DMAed in chunks so the first score
    matmul issues early
"""

import ml_dtypes
import numpy as np

import bass_rust
import concourse.bass as bass
import concourse.tile as tile
from concourse import mybir
from concourse.bass_utils import run_bass_kernel_spmd
from concourse.vector_clock import ScopedClock

# ---- problem constants (hardcoded; kernel.py must be self-contained) ----
B = 2
L = 2048
DIM = 1024
NUM_HEADS = 16
HEAD_DIM = 64
SCALE = HEAD_DIM ** -0.5

NCORES = 8
NH = 4             # heads per core
NPAIR = 2          # head pairs per core
C = NH * HEAD_DIM  # 256 head-cols per core
DA = HEAD_DIM + 1  # V augmented with ones column
KC = DIM // 128    # 8 contraction chunks for qkv proj
JC = L // 128      # 16 key-position chunks
NIC = 4            # query chunks of 512
ICW = 512          # query chunk width

F32 = mybir.dt.float32
F32R = mybir.dt.float32r
BF16 = mybir.dt.bfloat16

# walrus in this container rejects >4 sync waits on one CTRL (drain)
# instruction; split the final TileContext drain into multiple drains.
_MAX_DRAIN_WAITS = 1


def _wait_limit(inst):
    # walrus struct wait-slot capacity varies by opcode; matmul (S3_LW)
    # and DMA structs only fit one sync wait. Use 1 everywhere for safety.
    return 1


def _merge_waits(base, extra):
    """Merge sem waits; same-sem waits collapse to the max wait value."""
    out = {w.id: w for w in base}
    for w in extra:
        cur = out.get(w.id)
        if cur is None or w.wait_value > cur.wait_value:
            out[w.id] = w
    return list(out.values())


def _fix_excess_waits(nc):
    """Walrus encodes at most 1 sync wait per instruction in this build.
    For instructions carrying more, insert ENGINE_NOP wait-carriers
    immediately before them on the same engine stream — semantically
    identical (waits execute at the same stream position)."""
    def make_nop(like_inst):
        eng = nc.engines[like_inst.engine]
        bi = eng.nop(nofuse=True, hint="waitsplit")
        nop = bi.ins if hasattr(bi, "ins") else bi
        # isa() appended it to the current (last) block; pull it out.
        for bb2 in nc.main_func.blocks:
            lst = bb2.instructions
            if lst and lst[-1] is nop:
                lst.pop()
                break
        return nop

    for bb in nc.main_func.blocks:
        insts = bb.instructions  # live list
        i = 0
        while i < len(insts):
            inst = insts[i]
            si = inst.sync_info
            lim = _wait_limit(inst)
            if si is None or not si.on_wait or len(si.on_wait) <= lim:
                i += 1
                continue
            waits = _merge_waits(list(si.on_wait), [])
            if len(waits) <= lim:
                inst.sync_info = bass_rust.SyncInfo(
                    on_wait=waits, on_update=list(si.on_update)
                )
                i += 1
                continue
            keep = waits[-lim:]
            overflow = waits[:-lim]
            for w in overflow:
                nop = make_nop(inst)
                nop.sync_info = bass_rust.SyncInfo(on_wait=[w], on_update=[])
                insts.insert(i, nop)
                i += 1
            inst.sync_info = bass_rust.SyncInfo(
                on_wait=keep, on_update=list(si.on_update)
            )
            i += 1


def _split_drain_and_barrier(self, tick_clock, wait_clock):
    _fix_excess_waits(self.nc)
    drain_inst = self.nc.sync.drain()
    wait_clock.add_sem_waits(
        drain_inst.ins, ScopedClock({None: tick_clock.global_clock})
    )
    si = drain_inst.ins.sync_info
    waits = list(si.on_wait) if si is not None and si.on_wait else []
    if len(waits) > _MAX_DRAIN_WAITS:
        drain_inst.ins.sync_info = bass_rust.SyncInfo(
            on_wait=waits[:_MAX_DRAIN_WAITS], on_update=list(si.on_update)
        )
        rest = waits[_MAX_DRAIN_WAITS:]
        while rest:
            d2 = self.nc.sync.drain()
            d2.ins.sync_info = bass_rust.SyncInfo(
                on_wait=rest[:_MAX_DRAIN_WAITS], on_update=[]
            )
            rest = rest[_MAX_DRAIN_WAITS:]
    self.nc.all_engine_barrier()
    assert self.sems is not None
    popped = self.nc._tile_sem_poison_stack.pop()
    assert popped is self._sem_poison
    # RANGE_CLEAR's count field can't encode large ranges; clear in chunks.
    sems = list(self.sems.allocated().values())
    for k in range(0, len(sems), 8):
        self.nc.clear_and_free_semaphores(sems[k:k + 8])
    self.nc.all_engine_barrier()


tile.TileContext._drain_and_barrier = _split_drain_and_barrier

# This walrus build allows at most 2 sync waits per instruction. Collapse
# all HWDGE DMA completions onto a single semaphore lane so consumers that
# wait on two different DMAed tiles (plus a slot release) stay within the
# limit.
import concourse.tile_sem_assignment as _tsa  # noqa: E402

_tsa.NUM_HWDGE_SEMS = 1


def _build_nc() -> bass.Bass:
    nc = bass.Bass("TRN2", target_bir_lowering=False, debug=False)

    # host-swizzled so each DMA chunk is contiguous per partition:
    # x4[p, lc, kc, l'] = x^T[kc*128+p, lc*512+l']
    x4_h = nc.dram_tensor("x4", [128, 4, KC, 512], BF16, kind="ExternalInput")
    # w6[p, seg, kc, c]; seg = pair*3 + {0:K, 1:Q, 2:V}
    w6_h = nc.dram_tensor("w6", [128, 6, KC, 128], BF16, kind="ExternalInput")
    # wp2[p, cc, o] = w_proj^T[cc*128+p, o]
    wp2_h = nc.dram_tensor("wp2", [128, 2, DIM], BF16, kind="ExternalInput")
    ones_h = nc.dram_tensor("ones", [128, 32], BF16, kind="ExternalInput")
    onesr_h = nc.dram_tensor("onesr", [1, 64], F32, kind="ExternalInput")
    out_h = nc.dram_tensor("out", [L, DIM], F32, kind="ExternalOutput")

    with tile.TileContext(nc) as tc:
        import contextlib
        with contextlib.ExitStack() as ctx:
            singles = ctx.enter_context(tc.tile_pool(name="singles", bufs=1))
            e_pool = ctx.enter_context(tc.tile_pool(name="e", bufs=3))
            dpool = ctx.enter_context(tc.tile_pool(name="dp", bufs=2))
            out_pool = ctx.enter_context(tc.tile_pool(name="outp", bufs=4))

            wT_sb = singles.tile([128, 6, KC, 128], BF16)
            wp_sb = singles.tile([128, 2, DIM], BF16)
            ones_sb = singles.tile([128, 32], BF16)
            onesr_sb = singles.tile([1, 64], F32)
            # x^T chunks: one tile per l-chunk of 512
            xt = [singles.tile([128, KC, 512], BF16, name=f"xt{lc}",
                               tag=f"xt{lc}") for lc in range(4)]
            # Q^T / K^T per pair: [128 (2 heads x 64 dims), L]
            qt = [singles.tile([128, L], BF16, name=f"qt{p}", tag=f"qt{p}")
                  for p in range(NPAIR)]
            kt = [singles.tile([128, L], BF16, name=f"kt{p}", tag=f"kt{p}")
                  for p in range(NPAIR)]
            # V natural layout + ones col, per pair
            vt = [singles.tile([128, JC, 2, DA], BF16, name=f"vt{p}",
                               tag=f"vt{p}") for p in range(NPAIR)]
            # normalized O^T as lhsT for the out-proj; per query chunk
            po = [singles.tile([128, NPAIR, ICW], BF16, name=f"po{i}",
                               tag=f"po{i}") for i in range(NIC)]

            # ---- input DMAs (sync HWDGE ring, FIFO): prefix needs
            # w[K0,Q0,V0] + xt0 first; all chunks contiguous/partition ----
            nc.sync.dma_start(wT_sb[:, 0], w6_h[:, 0])   # K0
            nc.sync.dma_start(xt[0][:], x4_h[:, 0])
            nc.sync.dma_start(wT_sb[:, 1], w6_h[:, 1])   # Q0
            nc.sync.dma_start(wT_sb[:, 2], w6_h[:, 2])   # V0
            nc.sync.dma_start(ones_sb[:], ones_h[:])
            nc.sync.dma_start(onesr_sb[:], onesr_h[:])
            nc.sync.dma_start(xt[1][:], x4_h[:, 1])
            nc.sync.dma_start(wT_sb[:, 3], w6_h[:, 3])
            nc.sync.dma_start(wT_sb[:, 4], w6_h[:, 4])
            nc.sync.dma_start(wT_sb[:, 5], w6_h[:, 5])
            nc.sync.dma_start(xt[2][:], x4_h[:, 2])
            nc.sync.dma_start(xt[3][:], x4_h[:, 3])
            nc.sync.dma_start(wp_sb[:], wp2_h[:])

            # ones columns for V (denominator trick), via cheap DVE copy
            for p in range(NPAIR):
                nc.vector.tensor_copy(
                    vt[p][:, :, :, 64:65],
                    ones_sb[:].rearrange("q (a b) -> q a b", a=JC)[:, :, :, None],
                )

            with tc.tile_pool(name="psS", bufs=2, space="PSUM") as psS_pool, \
                 tc.tile_pool(name="psO", bufs=1, space="PSUM") as psO_pool, \
                 tc.tile_pool(name="scr", bufs=2, space="PSUM") as scr:

                # ---- stage-1 units (emitted inline or as filler) ----
                def kq_unit(p, lc, which):  # which: 0=Q, 1=K
                    seg = p * 3 + (1 if which == 0 else 0)
                    ps = scr.tile([128, 512], F32, tag="scr")
                    for kc in range(KC):
                        nc.tensor.matmul(
                            ps[:],
                            wT_sb[:, seg, kc, :],
                            xt[lc][:, kc, :],
                            start=(kc == 0),
                            stop=(kc == KC - 1),
                        )
                    dst = qt[p] if which == 0 else kt[p]
                    nc.vector.tensor_copy(
                        dst[:, lc * 512:(lc + 1) * 512], ps[:]
                    )

                def v_unit(p, lc):
                    seg = p * 3 + 2
                    for lt in range(4):
                        psv = scr.tile([128, 128], F32, tag="scr")
                        for kc in range(KC):
                            nc.tensor.matmul(
                                psv[:],
                                xt[lc][:, kc, lt * 128:(lt + 1) * 128],
                                wT_sb[:, seg, kc, :],
                                start=(kc == 0),
                                stop=(kc == KC - 1),
                            )
                        jc = lc * 4 + lt
                        nc.vector.tensor_copy(
                            vt[p][:, jc, :, 0:64],
                            psv[:].rearrange("q (hh d) -> q hh d", hh=2),
                        )

                def proj_tile(ic, it):
                    ot = out_pool.tile([128, DIM], F32, tag="outp")
                    for oc in range(2):
                        psP = scr.tile([128, 512], F32, tag="scr")
                        for cc in range(2):
                            nc.tensor.matmul(
                                psP[:],
                                po[ic][:, cc, it * 128:(it + 1) * 128],
                                wp_sb[:, cc, oc * 512:(oc + 1) * 512],
                                start=(cc == 0),
                                stop=(cc == 1),
                            )
                        nc.vector.tensor_copy(
                            ot[:, oc * 512:(oc + 1) * 512], psP[:]
                        )
                    l0 = ic * ICW + it * 128
                    nc.sync.dma_start(out_h[l0:l0 + 128, :], ot[:])

                fq = []  # filler queue: closures emitting PE work

                def fill(n=1):
                    for _ in range(n):
                        if fq:
                            fq.pop(0)()

                # ---- prefix: pair-0 lc0 projections ----
                kq_unit(0, 0, 1)   # K0(lc0)
                kq_unit(0, 0, 0)   # Q0(lc0)
                v_unit(0, 0)       # V0(lc0)



                def attention_pair(p, sched=None, on_normalize=None):
                    sched = sched or {}
                    psO = [None, None]
                    e_tiles = {}

                    def scores_block(ic, jj):
                        # both heads of the pair: concurrent row-tiled MMs
                        psS = psS_pool.tile([128, 2, 512], F32, name="psS",
                                            tag="psS")
                        for hh in range(2):
                            nc.tensor.matmul(
                                psS[:, hh, :],
                                kt[p][hh * 64:(hh + 1) * 64,
                                      jj * 128:(jj + 1) * 128],
                                qt[p][hh * 64:(hh + 1) * 64,
                                      ic * ICW:(ic + 1) * ICW],
                                start=True,
                                stop=True,
                            )
                        e = e_pool.tile([128, 2, 512], BF16, tag="e")
                        nc.scalar.activation(
                            e[:], psS[:],
                            mybir.ActivationFunctionType.Exp,
                            scale=float(SCALE),
                        )
                        e_tiles[(ic, jj)] = e

                    def pv_block(ic, jj):
                        e = e_tiles.pop((ic, jj))
                        for hh in range(2):
                            nc.tensor.matmul(
                                psO[hh][:],
                                vt[p][:, jj, hh, :],
                                e[:, hh, :],
                                start=(jj == 0),
                                stop=(jj == JC - 1),
                            )

                    def normalize(ic):
                        # denominators of both heads into one [1, 1024]
                        # fp32 row; one fast approx reciprocal; two K=1
                        # matmuls broadcast 1/d to the partition halves.
                        d = dpool.tile([1, 1024], F32, tag="d")
                        for hh in range(2):
                            nc.vector.tensor_copy(
                                d[0:1, hh * 512:(hh + 1) * 512],
                                psO[hh][64:65, :],
                            )
                            nc.vector.tensor_copy(
                                po[ic][hh * 64:(hh + 1) * 64, p, :],
                                psO[hh][0:64, :],
                            )
                        rb = scr.tile([128, 512], F32, tag="scr")
                        for hh in range(2):
                            nc.tensor.matmul(
                                rb[hh * 64:(hh + 1) * 64, :],
                                onesr_sb[:],
                                d[0:1, hh * 512:(hh + 1) * 512],
                                start=True,
                                stop=True,
                            )
                        rr = dpool.tile([128, 512], F32, tag="r")
                        nc.vector.reciprocal(rr[:], rb[:])
                        nc.vector.tensor_mul(
                            po[ic][:, p, :], po[ic][:, p, :], rr[:]
                        )

                    for bt in range(NIC * JC + 1):
                        ic, t = divmod(bt, JC)
                        if bt < NIC * JC:
                            if t == 0:
                                psO[0] = psO_pool.tile(
                                    [DA, 512], F32, name="psOA", tag="psOA")
                                psO[1] = psO_pool.tile(
                                    [DA, 512], F32, name="psOB", tag="psOB")
                            scores_block(ic, t)
                        if bt > 0:
                            pic, pt = divmod(bt - 1, JC)
                            pv_block(pic, pt)
                            if pt == JC - 1:
                                normalize(pic)
                                if on_normalize is not None:
                                    on_normalize(pic)
                        for task in sched.get(bt, ()):
                            task()

                # ---- pair 0 attention: schedule keyed by block index.
                # K0(lc) consumed at bt=4lc; V0(lc) at bt=4lc+1; Q0(ic) at
                # bt=16ic. Pair-1 units have no deadline inside p0. ----
                sched0 = {
                    1: [lambda: kq_unit(0, 1, 1)],
                    2: [lambda: v_unit(0, 1)],
                    5: [lambda: kq_unit(0, 2, 1)],
                    6: [lambda: v_unit(0, 2)],
                    9: [lambda: kq_unit(0, 3, 1)],
                    10: [lambda: v_unit(0, 3)],
                    13: [lambda: kq_unit(0, 1, 0)],
                    20: [lambda: kq_unit(0, 2, 0)],
                    24: [lambda: kq_unit(1, 0, 1)],
                    28: [lambda: kq_unit(1, 1, 1)],
                    32: [lambda: kq_unit(1, 2, 1)],
                    36: [lambda: kq_unit(1, 3, 1)],
                    40: [lambda: kq_unit(0, 3, 0)],
                    44: [lambda: kq_unit(1, 0, 0)],
                    47: [lambda: v_unit(1, 0)],
                    50: [lambda: v_unit(1, 1)],
                    53: [lambda: v_unit(1, 2)],
                    56: [lambda: v_unit(1, 3)],
                    59: [lambda: kq_unit(1, 1, 0)],
                    61: [lambda: kq_unit(1, 2, 0)],
                    63: [lambda: kq_unit(1, 3, 0)],
                }
                attention_pair(0, sched=sched0)

                # ---- pair 1 attention; proj(ic) legal after pair-1
                # normalize(ic), which is emitted at the start of block
                # 16*ic+16. Spread the 4 tiles over the next chunk. ----
                sched1 = {}
                for pic in range(NIC):
                    for it in range(4):
                        bt = 16 * pic + 17 + 3 * it
                        sched1.setdefault(min(bt, NIC * JC), []).append(
                            lambda ic=pic, it=it: proj_tile(ic, it)
                        )
                attention_pair(1, sched=sched1)
    return nc


_NC_CACHE = None


def _get_nc():
    global _NC_CACHE
    if _NC_CACHE is None:
        _NC_CACHE = _build_nc()
    return _NC_CACHE


def kernel(x, w_qkv, w_proj, b_proj, _trace=False):
    x = np.asarray(x, dtype=np.float32)
    w_qkv = np.asarray(w_qkv, dtype=np.float32)
    w_proj = np.asarray(w_proj, dtype=np.float32)
    b_proj = np.asarray(b_proj, dtype=np.float32)

    nc = _get_nc()
    in_maps = []
    for b in range(B):
        xT = x[b].T  # [DIM, L]
        # x4[p, lc, kc, l'] = xT[kc*128+p, lc*512+l']
        x4 = np.ascontiguousarray(
            xT.reshape(KC, 128, 4, 512).transpose(1, 2, 0, 3)
        ).astype(ml_dtypes.bfloat16)
        for hg in range(4):
            s = C * hg
            segs = []
            for p in range(NPAIR):
                o = s + p * 128
                segs.append(w_qkv[DIM + o:DIM + o + 128])      # K pair p
                segs.append(w_qkv[o:o + 128])                  # Q pair p
                segs.append(w_qkv[2 * DIM + o:2 * DIM + o + 128])  # V
            w_cat = np.concatenate(segs, 0)  # [768, 1024]
            # w6[p, seg, kc, c] = w_cat[seg*128+c, kc*128+p]
            w6 = np.ascontiguousarray(
                w_cat.reshape(6, 128, KC, 128).transpose(3, 0, 2, 1)
            ).astype(ml_dtypes.bfloat16)
            # wp2[p, cc, o] = w_proj[o, s+cc*128+p]
            wp2 = np.ascontiguousarray(
                w_proj[:, s:s + C].T.reshape(2, 128, DIM).transpose(1, 0, 2)
            ).astype(ml_dtypes.bfloat16)
            in_maps.append({
                "x4": x4,
                "w6": w6,
                "wp2": wp2,
                "ones": np.ones((128, 32), ml_dtypes.bfloat16),
                "onesr": np.ones((1, 64), np.float32),
            })

    res = run_bass_kernel_spmd(nc, in_maps, list(range(NCORES)), trace=_trace)
    parts = [res.results[i]["out"] for i in range(NCORES)]
    out = np.stack([
        parts[0] + parts[1] + parts[2] + parts[3],
        parts[4] + parts[5] + parts[6] + parts[7],
    ]).astype(np.float32) + b_proj[None, None, :].astype(np.float32)
    if _trace:
        return out, res
    return out
